# revision 18
# baseline (speedup 1.0000x reference)
"""MixIT loss kernel for Trainium2 (raw Bass), 8-way data-parallel over batch.

Math: the loss only depends on the 10x10 Gram matrix of the stacked signals
D = [sources(8); mixtures(2)] over T=32000:
  noise_energy[k,j] = ||x_j - sum_{m in S_kj} s_m||^2
expands into entries of G = D @ D.T.  With a_k = assignment row of mixture 1:
  d1_k = ne1_k + tau*E1 = E1*(1+tau) - 2*a_k.C1 + a_k G a_k
  d0_k = ne0_k + tau*E0 = (E0*(1+tau) - 2*sum(C0) + sum(G)) + 2*a_k.(C0-h) + a_k G a_k
  per_sample = 10/ln(10) * (ln(min_k d1_k*d0_k) - ln(E0*E1))

Dataflow per core (one batch sample per core, host averages 8 scalars):
  1. One fat contiguous DMA per wave loads the natural layout
     nat[p, s*250 + n] = D[s, p*250 + n] (1000B runs, full DMA bandwidth).
  2. DVE + GPSIMD copies repack to R[p, b*100 + s*10 + i] (n = b*10 + i) so
     every contiguous 100-column block holds all 10 signals interleaved.
  3. 25 PE matmuls (each block against itself) accumulate a 100x100 PSUM
     whose (i == i') 10x10 sub-grid holds partial Grams; 10 selector matmuls
     (lhsT = identity[:, i::10]) fold them into G10 (compute engines cannot
     address partition offset/stride 10 directly, PE contraction can).
  4. Tiny matmuls + DVE ops compute the 254 per-combo noise energies, the
     min over combos and the log -- all on one partition.

Raw Bass (not Tile): this toolchain's codegen allows a single sync-wait slot
per instruction, so all cross-engine waits are standalone wait_ge
instructions and each engine runs a hand-scheduled program.
"""

import itertools
from contextlib import ExitStack

import numpy as np

from concourse import bass, mybir
from concourse.bass_utils import run_bass_kernel_spmd

F32 = mybir.dt.float32

B = 8
M = 8  # sources
NMIX = 2
NSIG = M + NMIX  # 10 signals stacked: sources then mixtures
T = 32000
P = 128
NCHUNK = T // P  # 250 elements per partition per signal
LBLK = 10  # i-values per Gram block (10*10 = 100 <= 128 stationary cols)
NBLK = NCHUNK // LBLK  # 25 Gram blocks
BW = NSIG * LBLK  # 100 columns per Gram block
K = 2**M - 2  # 254 assignment combos
TAU = 1e-6
LOG10_SCALE = 10.0 / float(np.log(10.0))

WAVE_EDGES = [0, 12, 25]  # Gram-block ranges per DMA wave
N_WAVES = len(WAVE_EDGES) - 1


def _const_matrix() -> np.ndarray:
    """[100, 355] f32: [identity(100) | ones10 column | ones254 row]."""
    c = np.zeros((BW, BW + 1 + K), dtype=np.float32)
    c[:BW, :BW] = np.eye(BW, dtype=np.float32)
    c[:M, BW] = 1.0
    c[0, BW + 1 :] = 1.0
    return c


def _assignment_matrix() -> np.ndarray:
    """[M, K] f32: a1t[m, k] = 1 if source m goes to mixture 1 under combo k.

    Same enumeration as the reference's build_A_system (order irrelevant: the
    loss takes a min over k).
    """
    cols = [a for a in itertools.product([0, 1], repeat=M) if 0 < sum(a) < M]
    return np.array(cols, dtype=np.float32).T.copy()


def _build_kernel() -> bass.Bass:
    nc = bass.Bass(trn_type="TRN2")
    data = nc.declare_dram_parameter("data", [NSIG, T], F32, isOutput=False)
    a1t = nc.declare_dram_parameter("a1t", [M, K], F32, isOutput=False)
    cst = nc.declare_dram_parameter("cst", [BW, BW + 1 + K], F32, isOutput=False)
    out = nc.declare_dram_parameter("loss", [1, 1], F32, isOutput=True)

    with ExitStack() as ctx:
        sb = lambda name, shape: ctx.enter_context(nc.sbuf_tensor(name, shape, F32))
        ps = lambda name, shape: ctx.enter_context(nc.psum_tensor(name, shape, F32))

        nat = sb("nat", [P, NSIG * NCHUNK])
        rint = sb("rint", [P, NSIG * NCHUNK])
        csb = sb("csb", [BW, BW + 1 + K])
        a1sb = sb("a1sb", [M, K])
        va1 = sb("va1", [M, K])
        pc = sb("pc", [BW, BW])
        g10 = sb("g10", [NSIG, NSIG])
        rowsb = sb("rowsb", [1, 3 * NSIG])
        hsb = sb("hsb", [NSIG, 1])
        sg = sb("sg", [1, 1])
        e1s = sb("e1s", [1, 1])
        s1 = sb("s1", [1, 1])
        s2 = sb("s2", [1, 1])
        e0s = sb("e0s", [1, 1])
        c1m2 = sb("c1m2", [M, 1])
        c0x2 = sb("c0x2", [M, 1])
        c0m2 = sb("c0m2", [M, 1])
        prod = sb("prod", [M, K])
        d0c = sb("d0c", [1, K])
        pk = sb("pk", [1, K])
        mn = sb("mn", [1, 1])
        ee = sb("ee", [1, 1])
        rec = sb("rec", [1, 1])
        q = sb("q", [1, 1])
        lg = sb("lg", [1, 1])
        loss = sb("loss_t", [1, 1])

        gp = ps("gp", [BW, BW])
        g10p = ps("g10p", [NSIG, NSIG])
        rowp = ps("rowp", [1, 3 * NSIG])
        hcol = ps("hcol", [NSIG, 1])
        qt = ps("qt", [M, K])
        ne1 = ps("ne1", [1, K])
        ne0 = ps("ne0", [1, K])

        # access-pattern views
        datav = data[:, :].rearrange("s (p n) -> p s n", p=P)
        natv = nat[:, :].rearrange("p (s n) -> p s n", s=NSIG)
        natr = nat[:, :].rearrange("p (s b i) -> p b s i", s=NSIG, i=LBLK)
        rr = rint[:, :].rearrange("p (b s i) -> p b s i", s=NSIG, i=LBLK)

        dsem0 = ctx.enter_context(nc.semaphore("dsem0"))  # a1t + cst
        dsem_w = [
            ctx.enter_context(nc.semaphore(f"dsem_w{w}")) for w in range(N_WAVES)
        ]
        dsem_out = ctx.enter_context(nc.semaphore("dsem_out"))
        pe_sem = ctx.enter_context(nc.semaphore("pe_sem"))
        dve_sem = ctx.enter_context(nc.semaphore("dve_sem"))
        pool_sem = ctx.enter_context(nc.semaphore("pool_sem"))
        act_sem = ctx.enter_context(nc.semaphore("act_sem"))
        block = ctx.enter_context(nc.Block())

        e0_ap = rowsb[0:1, M : M + 1]                     # G10[8,8]
        e1_ap = rowsb[0:1, NSIG + M + 1 : NSIG + M + 2]   # G10[9,9]
        sumc0_ap = rowsb[0:1, 2 * NSIG + M : 2 * NSIG + M + 1]
        id100 = csb[:, 0:BW]
        ones10 = csb[0:NSIG, BW : BW + 1]
        ones254 = csb[0:1, BW + 1 : BW + 1 + K]
        e8col = csb[0:NSIG, M : M + 1]
        e9col = csb[0:NSIG, M + 1 : M + 2]

        @block.sync
        def _(sync):
            sync.dma_start(out=a1sb[:, :], in_=a1t[:, :]).then_inc(dsem0, 16)
            sync.dma_start(out=csb[:, :], in_=cst[:, :]).then_inc(dsem0, 16)
            for w in range(N_WAVES):
                n0 = WAVE_EDGES[w] * LBLK
                n1 = WAVE_EDGES[w + 1] * LBLK
                sync.dma_start(
                    out=natv[:, :, n0:n1], in_=datav[:, :, n0:n1]
                ).then_inc(dsem_w[w], 16)
            sync.wait_ge(dve_sem, 23)
            sync.dma_start(out=out[:, :], in_=loss[:, :]).then_inc(dsem_out, 16)
            sync.wait_ge(dsem_out, 16)

        @block.gpsimd
        def _(gpsimd):
            for w in range(N_WAVES):
                b0, b1 = WAVE_EDGES[w], WAVE_EDGES[w + 1]
                bm = (b0 + b1) // 2
                gpsimd.wait_ge(dsem_w[w], 16)
                gpsimd.tensor_copy(rr[:, bm:b1], natr[:, bm:b1]).then_inc(pool_sem, 1)

        @block.vector
        def _(vector):
            # Same-engine RAW needs explicit sem sync on this HW: every DVE
            # op increments dve_sem; dependent ops wait on the running count.
            cnt = [0]

            def bump(ins):
                ins.then_inc(dve_sem, 1)
                cnt[0] += 1

            def chain():
                if cnt[0]:
                    vector.wait_ge(dve_sem, cnt[0])

            vector.wait_ge(dsem0, 32)
            bump(vector.tensor_copy(va1[:, :], a1sb[:, :]))               # 1
            for w in range(N_WAVES):
                b0, b1 = WAVE_EDGES[w], WAVE_EDGES[w + 1]
                bm = (b0 + b1) // 2
                vector.wait_ge(dsem_w[w], 16)
                bump(vector.tensor_copy(rr[:, b0:bm], natr[:, b0:bm]))    # 2,3
            vector.wait_ge(pe_sem, NBLK)
            bump(vector.tensor_copy(pc[:, :], gp[:, :]))                  # 4
            vector.wait_ge(pe_sem, NBLK + LBLK)
            bump(vector.tensor_copy(g10[:, :], g10p[:, :]))               # 5
            vector.wait_ge(pe_sem, NBLK + LBLK + 4)  # hcol + 3 rowp done
            bump(vector.tensor_copy(rowsb[:, :], rowp[:, :]))             # 6
            bump(vector.tensor_copy(hsb[:, :], hcol[:, :]))               # 7
            chain()
            bump(vector.reduce_sum(
                sg[:, :],
                rowsb[0:1, 2 * NSIG : 2 * NSIG + M],
                axis=mybir.AxisListType.X,
            ))                                                            # 8
            chain()
            bump(vector.tensor_scalar_mul(e1s[:, :], e1_ap, 1.0 + TAU))  # 9
            chain()
            bump(vector.tensor_scalar_mul(s1[:, :], e0_ap, 1.0 + TAU))   # 10
            chain()
            bump(vector.scalar_tensor_tensor(
                s2[:, :], sumc0_ap, -2.0, s1[:, :],
                op0=mybir.AluOpType.mult, op1=mybir.AluOpType.add,
            ))                                                            # 11
            chain()
            bump(vector.tensor_add(e0s[:, :], s2[:, :], sg[:, :]))        # 12
            chain()
            bump(vector.tensor_scalar_mul(c1m2[:, :], g10[0:M, M + 1 : M + 2], -2.0))  # 13
            chain()
            bump(vector.tensor_scalar_mul(c0x2[:, :], g10[0:M, M : M + 1], 2.0))       # 14
            chain()
            bump(vector.scalar_tensor_tensor(
                c0m2[:, :], hsb[0:M, 0:1], -2.0, c0x2[:, :],
                op0=mybir.AluOpType.mult, op1=mybir.AluOpType.add,
            ))                                                            # 15
            vector.wait_ge(pe_sem, NBLK + LBLK + 5)  # qt
            chain()
            bump(vector.tensor_mul(prod[:, :], qt[:, :], va1[:, :]))      # 16
            vector.wait_ge(pe_sem, NBLK + LBLK + 11)  # ne1 + ne0
            bump(vector.tensor_copy(d0c[:, :], ne0[:, :]))                # 17
            chain()
            bump(vector.tensor_mul(pk[:, :], ne1[:, :], d0c[:, :]))       # 18
            chain()
            bump(vector.tensor_reduce(
                mn[:, :], pk[:, :], axis=mybir.AxisListType.X, op=mybir.AluOpType.min
            ))                                                            # 19
            chain()
            bump(vector.tensor_mul(ee[:, :], e0_ap, e1_ap))               # 20
            chain()
            bump(vector.reciprocal(rec[:, :], ee[:, :]))                  # 21
            chain()
            bump(vector.tensor_mul(q[:, :], mn[:, :], rec[:, :]))         # 22
            vector.wait_ge(act_sem, 1)
            bump(vector.tensor_scalar_mul(loss[:, :], lg[:, :], LOG10_SCALE))  # 23

        @block.scalar
        def _(scalar):
            scalar.wait_ge(dve_sem, 22)
            scalar.activation(
                lg[:, :], q[:, :], mybir.ActivationFunctionType.Ln
            ).then_inc(act_sem, 1)

        @block.tensor
        def _(tensor):
            for w in range(N_WAVES):
                b0, b1 = WAVE_EDGES[w], WAVE_EDGES[w + 1]
                tensor.wait_ge(dve_sem, 1 + (w + 1))
                tensor.wait_ge(pool_sem, w + 1)
                for blk in range(b0, b1):
                    cols = rint[:, blk * BW : (blk + 1) * BW]
                    tensor.matmul(
                        gp[:, :],
                        cols,
                        cols,
                        start=(blk == 0),
                        stop=(blk == NBLK - 1),
                    ).then_inc(pe_sem, 1)
            tensor.wait_ge(dsem0, 32)  # consts (identity, ones) landed
            tensor.wait_ge(dve_sem, 4)  # pc copied
            for i in range(LBLK):
                tensor.matmul(
                    g10p[:, :],
                    id100[:, i :: LBLK],
                    pc[:, i :: LBLK],
                    start=(i == 0),
                    stop=(i == LBLK - 1),
                ).then_inc(pe_sem, 1)
            tensor.wait_ge(dve_sem, 5)  # g10 copied
            tensor.matmul(hcol[:, :], g10[:, :], ones10[:, :]).then_inc(pe_sem, 1)
            tensor.matmul(
                rowp[0:1, 0:NSIG], e8col, g10[:, :]
            ).then_inc(pe_sem, 1)
            tensor.matmul(
                rowp[0:1, NSIG : 2 * NSIG], e9col, g10[:, :]
            ).then_inc(pe_sem, 1)
            tensor.matmul(
                rowp[0:1, 2 * NSIG : 3 * NSIG], ones10[:, :], g10[:, :]
            ).then_inc(pe_sem, 1)
            tensor.matmul(qt[:, :], g10[0:M, 0:M], va1[:, :]).then_inc(pe_sem, 1)
            tensor.wait_ge(dve_sem, 16)  # prod + scaled columns ready
            tensor.matmul(
                ne1[:, :], ones10[0:M, :], prod[:, :], start=True, stop=False
            ).then_inc(pe_sem, 1)
            tensor.matmul(
                ne1[:, :], c1m2[:, :], va1[:, :], start=False, stop=False
            ).then_inc(pe_sem, 1)
            tensor.matmul(
                ne1[:, :], e1s[:, :], ones254[:, :], start=False, stop=True
            ).then_inc(pe_sem, 1)
            tensor.matmul(
                ne0[:, :], ones10[0:M, :], prod[:, :], start=True, stop=False
            ).then_inc(pe_sem, 1)
            tensor.matmul(
                ne0[:, :], c0m2[:, :], va1[:, :], start=False, stop=False
            ).then_inc(pe_sem, 1)
            tensor.matmul(
                ne0[:, :], e0s[:, :], ones254[:, :], start=False, stop=True
            ).then_inc(pe_sem, 1)

    return nc


_NC_CACHE: bass.Bass | None = None


def kernel(estimated_sources: np.ndarray, input_mixtures: np.ndarray) -> np.ndarray:
    global _NC_CACHE
    assert estimated_sources.shape == (B, M, T)
    assert input_mixtures.shape == (B, NMIX, T)
    if _NC_CACHE is None:
        _NC_CACHE = _build_kernel()
    nc = _NC_CACHE

    a1 = _assignment_matrix()
    cst = _const_matrix()
    est = np.asarray(estimated_sources, dtype=np.float32)
    mx = np.asarray(input_mixtures, dtype=np.float32)
    in_maps = [
        {
            "data": np.concatenate([est[b], mx[b]], axis=0),
            "a1t": a1,
            "cst": cst,
        }
        for b in range(B)
    ]
    res = run_bass_kernel_spmd(nc, in_maps, core_ids=list(range(B)))
    vals = np.array([res.results[b]["loss"][0, 0] for b in range(B)], dtype=np.float32)
    return np.asarray(vals.mean(), dtype=np.float32)
